# revision 22
# baseline (speedup 1.0000x reference)
"""Causal attention with ALiBi for B=1, T=4096, C=1024, H=16 on 8 TRN2 NeuronCores.

Sharding: tensor-parallel over heads. Core c computes heads {c, 8+c}:
 - slot A = head c (steep slope): i-major over 128-query tiles, window of
   W_A=4 key tiles (keys farther than ~385 positions contribute < 1e-6 of
   softmax mass for every head in this slot). Exps batched 8 same-distance
   blocks (1024 queries) wide with a per-partition ALiBi bias column.
 - slot B = head 8+c (shallow slope): full causal window, i-major over
   1024-query windows with the ALiBi bias referenced to the window's last
   row so every live exponent stays in range; far keys underflow to 0.

All matmuls run in bf16 (S, PV, QKV, output projection). Slot A S-matmuls
(contraction 64, PE rows 0-63) and slot B S-matmuls (rows 64-127) are
emitted adjacently so the PE array runs them concurrently via row tiling.
The causal mask is applied by multiplying the bf16 exp tiles with a 0/1
lower-triangle constant on DVE. P@V matmuls carry a ones column appended
to v so PSUM accumulates [O^T | l]; O^T is rescaled by 1/l (reciprocal +
partition broadcast) directly out of PSUM into a persistent bf16 O^T
tile. The output projection runs as a dense final phase; each core
writes a bf16 [T, C] partial of out = O^T.T @ Wo_slice, and the 8
partials are summed on the host (the TP all-reduce done at unshard time)
with bo.
"""

import math

import numpy as np

B, T, C, H = 1, 4096, 1024, 16
HD = C // H            # 64
NCORES = 8
P = 128
NTT = T // P           # 32 key/query tiles
NCT = C // P           # 8 contraction tiles
TBW = 512              # t-block width for QKV projection
NTB = T // TBW         # 8
GW = 1024              # attention group width (queries)
NG = T // GW           # 4 groups
W_A = 4                # slot A window in key tiles
SHIFT = 40.0           # uniform exponent shift (cancels in softmax)
QK_SCALE = 1.0 / math.sqrt(HD)


def get_slopes(n):
    def pow2(n):
        start = 2 ** (-(2 ** (-(math.log2(n) - 3))))
        return [start * (start ** i) for i in range(n)]
    if math.log2(n).is_integer():
        return pow2(n)
    cp2 = 2 ** math.floor(math.log2(n))
    return pow2(cp2) + get_slopes(2 * cp2)[0::2][: n - cp2]


_CACHE = {}


def _build(debug=False, loop_n=0, dump=False, skip_out=False, skip_attn=False,
           skip_a=False, skip_b=False):
    key = ("nc", debug, loop_n, dump, skip_out, skip_attn, skip_a, skip_b)
    if key in _CACHE:
        return _CACHE[key]

    import concourse.bacc as bacc
    import concourse.tile as tile
    from concourse import mybir

    f32 = mybir.dt.float32
    bf16 = mybir.dt.bfloat16
    ACT = mybir.ActivationFunctionType

    nc = bacc.Bacc(None, target_bir_lowering=False, debug=debug)

    xT = nc.dram_tensor("xT", [C, T], bf16, kind="ExternalInput")
    wq = nc.dram_tensor("wq", [C, P], bf16, kind="ExternalInput")
    wk = nc.dram_tensor("wk", [C, P], bf16, kind="ExternalInput")
    wv = nc.dram_tensor("wv", [C, P], bf16, kind="ExternalInput")
    bq = nc.dram_tensor("bq", [P, 1], f32, kind="ExternalInput")
    bk = nc.dram_tensor("bk", [P, 1], f32, kind="ExternalInput")
    bv = nc.dram_tensor("bv", [P, 1], f32, kind="ExternalInput")
    wo = nc.dram_tensor("wo", [P, C], bf16, kind="ExternalInput")
    aliA = nc.dram_tensor("aliA", [P, W_A], f32, kind="ExternalInput")
    aliB = nc.dram_tensor("aliB", [P, NTT], f32, kind="ExternalInput")
    triA = nc.dram_tensor("triA", [P, GW], bf16, kind="ExternalInput")
    out = nc.dram_tensor("out", [T, C], bf16, kind="ExternalOutput")
    if dump:
        dqT = nc.dram_tensor("dqT", [P, T], bf16, kind="ExternalOutput")
        dkT = nc.dram_tensor("dkT", [P, T], bf16, kind="ExternalOutput")
        dvS = nc.dram_tensor("dvS", [P, NTT * 130], bf16, kind="ExternalOutput")
        doT = nc.dram_tensor("doT", [P, T], bf16, kind="ExternalOutput")

    with tile.TileContext(nc) as tc:
        with tc.tile_pool(name="consts", bufs=1) as consts, \
             tc.tile_pool(name="big", bufs=1) as big, \
             tc.tile_pool(name="xt", bufs=24) as xt_pool, \
             tc.tile_pool(name="vt", bufs=3) as vt_pool, \
             tc.tile_pool(name="etA", bufs=3) as etA_pool, \
             tc.tile_pool(name="etB", bufs=3) as etB_pool, \
             tc.tile_pool(name="lr", bufs=2) as lr_pool, \
             tc.tile_pool(name="ob", bufs=4) as ob_pool, \
             tc.tile_pool(name="ps2", bufs=2, space="PSUM") as ps2_pool, \
             tc.tile_pool(name="poA", bufs=1, space="PSUM") as poA_pool, \
             tc.tile_pool(name="poB", bufs=1, space="PSUM") as poB_pool:

            # ---- constants
            wq_sb = consts.tile([P, NCT, P], bf16)
            wk_sb = consts.tile([P, NCT, P], bf16)
            wv_sb = consts.tile([P, NCT, P], bf16)
            nc.sync.dma_start(out=wq_sb, in_=wq.ap().rearrange("(t p) d -> p t d", p=P))
            nc.sync.dma_start(out=wk_sb, in_=wk.ap().rearrange("(t p) d -> p t d", p=P))
            nc.sync.dma_start(out=wv_sb, in_=wv.ap().rearrange("(t p) d -> p t d", p=P))
            wo_sb = consts.tile([P, C], bf16)
            nc.sync.dma_start(out=wo_sb, in_=wo[:, :])
            aliA_sb = consts.tile([P, W_A], f32)
            nc.sync.dma_start(out=aliA_sb, in_=aliA[:, :])
            aliB_sb = consts.tile([P, NTT], f32)
            nc.sync.dma_start(out=aliB_sb, in_=aliB[:, :])
            triA_sb = consts.tile([P, GW], bf16)
            nc.sync.dma_start(out=triA_sb, in_=triA[:, :])
            bq_sb = consts.tile([P, 1], f32)
            bk_sb = consts.tile([P, 1], f32)
            bv_sb = consts.tile([P, 1], f32)
            nc.sync.dma_start(out=bq_sb, in_=bq[:, :])
            nc.sync.dma_start(out=bk_sb, in_=bk[:, :])
            nc.sync.dma_start(out=bv_sb, in_=bv[:, :])
            zero_bf = consts.tile([P, TBW], bf16)
            nc.vector.memset(zero_bf, 0.0)
            ident = consts.tile([P, P], bf16)
            from concourse import masks as _masks
            _masks.make_identity(nc, ident)

            # ---- persistent activations
            # partitions 0-63: head A dims; 64-127: head B dims
            qT = big.tile([P, T], bf16)
            kT = big.tile([P, T], bf16)
            # per key tile: [0:64]=v_A dims, 64=ones_A, [65:129]=v_B, 129=ones_B
            # so lhsT_A = [:, jt, 0:65] puts o_A in PSUM rows 0:64 / l_A in 64,
            # and lhsT_B = [:, jt, 65:130] puts o_B in rows 0:64 / l_B in 64.
            vS = big.tile([P, NTT, 130], bf16)
            oT = big.tile([P, T], bf16)
            for jt in range(NTT):
                nc.vector.memset(vS[:, jt, 64:65], 1.0)
                nc.vector.memset(vS[:, jt, 129:130], 1.0)

            def qkv_tblock(tb):
                cols = slice(tb * TBW, (tb + 1) * TBW)
                xts = []
                for ct in range(NCT):
                    xt = xt_pool.tile([P, TBW], bf16, tag="xt")
                    nc.sync.dma_start(
                        out=xt, in_=xT[ct * P:(ct + 1) * P, cols])
                    xts.append(xt)
                for which, w_sb, b_sb in (("q", wq_sb, bq_sb),
                                          ("k", wk_sb, bk_sb),
                                          ("v", wv_sb, bv_sb)):
                    pp = ps2_pool.tile([P, TBW], f32, tag="ps2")
                    for ct in range(NCT):
                        nc.tensor.matmul(pp, lhsT=w_sb[:, ct], rhs=xts[ct],
                                         start=(ct == 0), stop=(ct == NCT - 1))
                    if which == "q":
                        nc.vector.tensor_scalar_add(qT[:, cols], pp, b_sb)
                    elif which == "k":
                        nc.vector.tensor_scalar_add(kT[:, cols], pp, b_sb)
                    else:
                        vt = vt_pool.tile([P, TBW], bf16, tag="vt")
                        nc.scalar.activation(out=vt, in_=pp, func=ACT.Identity,
                                             bias=b_sb, scale=1.0)
                        ptr = ps2_pool.tile([P, TBW], bf16, tag="ps2")
                        for q4 in range(4):
                            nc.tensor.transpose(
                                ptr[:, q4 * P:(q4 + 1) * P],
                                vt[:, q4 * P:(q4 + 1) * P], ident)
                        for q4 in range(4):
                            jt = 4 * tb + q4
                            # strided dest: dims 0:64 -> vS[..,0:64],
                            # dims 64:128 -> vS[..,65:129]
                            dst = vS[:, jt, :].rearrange(
                                "p (a b) -> p a b", a=2)[:, :, 0:64]
                            src = ptr[:, q4 * P:(q4 + 1) * P].rearrange(
                                "p (a b) -> p a b", a=2)
                            nc.vector.tensor_copy(out=dst, in_=src)

            def body():
                for og in range(NG):
                    # ---- QKV projections for this group's two t-blocks
                    qkv_tblock(2 * og)
                    qkv_tblock(2 * og + 1)
                    if skip_attn:
                        continue

                    # ---- attention for queries q0 .. q0+1024
                    q0 = og * GW
                    its = [8 * og + g for g in range(8)]

                    poA_t = poA_pool.tile([65, GW], f32, tag="poA")
                    poB_t = poB_pool.tile([65, GW], f32, tag="poB")
                    # open the poA banks: full-width zero matmuls set
                    # has_written everywhere so the ragged-start PV matmuls
                    # below can all accumulate with start=False
                    if not skip_a:
                        nc.tensor.matmul(poA_t[:, 0:TBW], lhsT=vS[:, 0, 0:65],
                                         rhs=zero_bf, start=True, stop=False,
                                         skip_group_check=True)
                        nc.tensor.matmul(poA_t[:, TBW:GW], lhsT=vS[:, 0, 0:65],
                                         rhs=zero_bf, start=True, stop=False,
                                         skip_group_check=True)

                    a_ds = [] if skip_a else list(
                        range(min(W_A - 1, 8 * og + 7), -1, -1))
                    b_jts = [] if skip_b else list(range(8 * og + 8))

                    def _flush_a(p, last):
                        d_, gs_, et_ = p
                        for g_ in gs_:
                            jt_ = its[g_] - d_
                            nc.tensor.matmul(poA_t[:, g_ * P:(g_ + 1) * P],
                                             lhsT=vS[:, jt_, 0:65],
                                             rhs=et_[:, g_ * P:(g_ + 1) * P],
                                             start=False,
                                             stop=(last and g_ == gs_[-1]),
                                             skip_group_check=True)

                    def _flush_b(p, last):
                        jt_, col0_, et_ = p
                        for lo, hi in ([(col0_, TBW), (TBW, GW)]
                                       if col0_ < TBW else [(col0_, GW)]):
                            nc.tensor.matmul(poB_t[:, lo:hi],
                                             lhsT=vS[:, jt_, 65:130],
                                             rhs=et_[:, lo:hi],
                                             start=(jt_ == 0), stop=last,
                                             skip_group_check=True)

                    pend_a = None
                    pend_b = None
                    for step in range(max(len(a_ds), len(b_jts))):
                        # emit both S matmuls adjacently: slot B uses PE rows
                        # 64-127, slot A rows 0-63, so the array runs them
                        # concurrently via row tiling
                        jt = dj = col0 = psB_t = None
                        d = gs = g0 = psA_t = None
                        if step < len(b_jts):
                            jt = b_jts[step]
                            dj = jt - 8 * og
                            col0 = 0 if dj < 0 else P * dj
                            psB_t = ps2_pool.tile([P, GW], f32, tag="ps2")
                            for lo, hi in ([(col0, TBW), (TBW, GW)]
                                           if col0 < TBW else [(col0, GW)]):
                                nc.tensor.matmul(
                                    psB_t[:, lo:hi],
                                    lhsT=kT[64:128, jt * P:(jt + 1) * P],
                                    rhs=qT[64:128, q0 + lo:q0 + hi],
                                    start=True, stop=True)
                        if step < len(a_ds):
                            d = a_ds[step]
                            gs = [g for g in range(8) if its[g] >= d]
                            g0 = min(gs)
                            psA_t = ps2_pool.tile([P, GW], f32, tag="ps2")
                            for g in gs:
                                nc.tensor.matmul(
                                    psA_t[:, g * P:(g + 1) * P],
                                    lhsT=kT[0:64, (its[g] - d) * P:(its[g] - d + 1) * P],
                                    rhs=qT[0:64, q0 + g * P:q0 + (g + 1) * P],
                                    start=True, stop=True)
                        if psB_t is not None:
                            etB_t = etB_pool.tile([P, GW], bf16, tag="etB")
                            nc.scalar.activation(
                                out=etB_t[:, col0:GW], in_=psB_t[:, col0:GW],
                                func=ACT.Exp,
                                bias=aliB_sb[:, 8 * og - jt + 7:8 * og - jt + 8],
                                scale=QK_SCALE)
                            if dj >= 0:
                                nc.vector.tensor_mul(
                                    etB_t[:, col0:col0 + P],
                                    etB_t[:, col0:col0 + P], triA_sb[:, 0:P])
                            if pend_b is not None:
                                _flush_b(pend_b, False)
                            pend_b = (jt, col0, etB_t)
                        if psA_t is not None:
                            etA_t = etA_pool.tile([P, GW], bf16, tag="etA")
                            nc.scalar.activation(
                                out=etA_t[:, g0 * P:GW], in_=psA_t[:, g0 * P:GW],
                                func=ACT.Exp, bias=aliA_sb[:, d:d + 1],
                                scale=QK_SCALE)
                            if d == 0:
                                nc.vector.tensor_mul(etA_t[:, g0 * P:GW],
                                                     etA_t[:, g0 * P:GW],
                                                     triA_sb[:, g0 * P:GW])
                            if pend_a is not None:
                                _flush_a(pend_a, False)
                            pend_a = (d, gs, etA_t)

                    if pend_a is not None:
                        _flush_a(pend_a, True)
                    if pend_b is not None:
                        _flush_b(pend_b, True)

                    # --- rescale O^T by 1/l straight out of PSUM
                    if not skip_a:
                        lrA = lr_pool.tile([1, GW], f32, tag="lrA")
                        nc.vector.reciprocal(lrA, poA_t[64:65, :])
                        lbA = lr_pool.tile([64, GW], f32, tag="lbA")
                        nc.gpsimd.partition_broadcast(lbA, lrA)
                        nc.vector.tensor_mul(oT[0:64, q0:q0 + GW],
                                             poA_t[0:64, :], lbA)
                    if not skip_b:
                        lrB = lr_pool.tile([1, GW], f32, tag="lrB")
                        nc.vector.reciprocal(lrB, poB_t[64:65, :])
                        lbB = lr_pool.tile([64, GW], f32, tag="lbB")
                        nc.gpsimd.partition_broadcast(lbB, lrB)
                        nc.vector.tensor_mul(oT[64:128, q0:q0 + GW],
                                             poB_t[0:64, :], lbB)

                if dump:
                    nc.sync.dma_start(out=dqT[:, :], in_=qT)
                    nc.sync.dma_start(out=dkT[:, :], in_=kT)
                    nc.sync.dma_start(out=dvS[:, :], in_=vS.rearrange("p a b -> p (a b)"))
                    nc.sync.dma_start(out=doT[:, :], in_=oT)

                # ---- output projection (dense final phase)
                for it in range(NTT if not skip_out else 0):
                    for eh in range(2):
                        ppo = ps2_pool.tile([P, TBW], f32, tag="ps2")
                        nc.tensor.matmul(ppo, lhsT=oT[:, it * P:(it + 1) * P],
                                         rhs=wo_sb[:, eh * TBW:(eh + 1) * TBW],
                                         start=True, stop=True)
                        ob = ob_pool.tile([P, TBW], bf16, tag="ob")
                        if eh % 2 == 0:
                            nc.vector.tensor_copy(out=ob, in_=ppo)
                        else:
                            nc.scalar.activation(out=ob, in_=ppo,
                                                 func=ACT.Identity, scale=1.0)
                        nc.sync.dma_start(
                            out=out[it * P:(it + 1) * P, eh * TBW:(eh + 1) * TBW],
                            in_=ob)

            # ---- optional timing loop wrapper
            import contextlib
            loop_ctx = tc.For_i(0, loop_n, 1) if loop_n else contextlib.nullcontext()
            with loop_ctx:
                body()

    nc.compile()
    _CACHE[key] = nc
    return nc


def shard_inputs(x, Wq, bq, Wk, bk, Wv, bv, Wo, bo):
    """Build the 8 per-core input maps."""
    import ml_dtypes
    x = np.asarray(x, dtype=np.float32)
    xT = np.ascontiguousarray(x.reshape(T, C).T).astype(ml_dtypes.bfloat16)
    slopes = get_slopes(H)
    jj = np.arange(P, dtype=np.float32)[:, None]          # partition index
    # 0/1 mask: valid iff query ii >= key jj, tiled GW/P wide
    tri01 = (jj <= jj.T).astype(np.float32)
    triA_np = np.tile(tri01, (1, GW // P)).astype(ml_dtypes.bfloat16)

    def col_slice(W, c):
        return np.ascontiguousarray(np.concatenate(
            [W[:, 64 * c:64 * c + 64], W[:, 64 * (8 + c):64 * (8 + c) + 64]],
            axis=1)).astype(ml_dtypes.bfloat16)

    def vec_slice(b, c):
        return np.ascontiguousarray(np.concatenate(
            [b[64 * c:64 * c + 64], b[64 * (8 + c):64 * (8 + c) + 64]])).reshape(P, 1)

    in_maps = []
    for c in range(NCORES):
        sA = np.float32(slopes[c])
        sB = np.float32(slopes[8 + c])
        dA = np.arange(W_A, dtype=np.float32)[None, :]
        aliA_np = (-sA * (128.0 * dA + 127.0 - jj) + SHIFT).astype(np.float32)
        # e = 8*og - jt in [-7, 24] -> column e + 7
        eB = np.arange(-7, NTT - 7, dtype=np.float32)[None, :]
        aliB_np = (-sB * (128.0 * eB + float(GW - 1) - jj) + SHIFT).astype(np.float32)
        in_maps.append({
            "xT": xT,
            "wq": col_slice(np.asarray(Wq, np.float32), c),
            "wk": col_slice(np.asarray(Wk, np.float32), c),
            "wv": col_slice(np.asarray(Wv, np.float32), c),
            "bq": vec_slice(np.asarray(bq, np.float32), c),
            "bk": vec_slice(np.asarray(bk, np.float32), c),
            "bv": vec_slice(np.asarray(bv, np.float32), c),
            "wo": np.ascontiguousarray(np.concatenate(
                [np.asarray(Wo, np.float32)[64 * c:64 * c + 64, :],
                 np.asarray(Wo, np.float32)[64 * (8 + c):64 * (8 + c) + 64, :]],
                axis=0)).astype(ml_dtypes.bfloat16),
            "aliA": aliA_np,
            "aliB": aliB_np,
            "triA": triA_np,
        })
    return in_maps


LAST_RESULT = None


def kernel(x, Wq, bq, Wk, bk, Wv, bv, Wo, bo, **run_kwargs):
    global LAST_RESULT
    from concourse.bass_utils import run_bass_kernel_spmd

    nc = _build()
    in_maps = shard_inputs(x, Wq, bq, Wk, bk, Wv, bv, Wo, bo)
    res = run_bass_kernel_spmd(nc, in_maps, core_ids=list(range(NCORES)), **run_kwargs)
    LAST_RESULT = res
    total = np.zeros((T, C), dtype=np.float32)
    for r in res.results:
        total += np.asarray(r["out"], dtype=np.float32)
    total += np.asarray(bo, np.float32)[None, :]
    return total.reshape(B, T, C)


# revision 23
# speedup vs baseline: 1.0757x; 1.0757x over previous
"""Causal attention with ALiBi for B=1, T=4096, C=1024, H=16 on 8 TRN2 NeuronCores.

Sharding: tensor-parallel over heads. Core c computes heads {c, 8+c}:
 - slot A = head c (steep slope): i-major over 128-query tiles, window of
   W_A=4 key tiles (keys farther than ~385 positions contribute < 1e-6 of
   softmax mass for every head in this slot). Exps batched 8 same-distance
   blocks (1024 queries) wide with a per-partition ALiBi bias column.
 - slot B = head 8+c (shallow slope): full causal window, i-major over
   1024-query windows with the ALiBi bias referenced to the window's last
   row so every live exponent stays in range; far keys underflow to 0.

All matmuls run in bf16 (S, PV, QKV, output projection). Slot A S-matmuls
(contraction 64, PE rows 0-63) and slot B S-matmuls (rows 64-127) are
emitted adjacently so the PE array runs them concurrently via row tiling.
The causal mask is applied by multiplying the bf16 exp tiles with a 0/1
lower-triangle constant on DVE. P@V matmuls carry a ones column appended
to v so PSUM accumulates [O^T | l]; O^T is rescaled by 1/l (reciprocal +
partition broadcast) directly out of PSUM into a persistent bf16 O^T
tile. The output projection runs as a dense final phase; each core
writes a bf16 [T, C] partial of out = O^T.T @ Wo_slice, and the 8
partials are summed on the host (the TP all-reduce done at unshard time)
with bo.
"""

import math

import numpy as np

B, T, C, H = 1, 4096, 1024, 16
HD = C // H            # 64
NCORES = 8
P = 128
NTT = T // P           # 32 key/query tiles
NCT = C // P           # 8 contraction tiles
TBW = 512              # t-block width for QKV projection
NTB = T // TBW         # 8
GW = 1024              # attention group width (queries)
NG = T // GW           # 4 groups
W_A = 4                # slot A window in key tiles
SHIFT = 40.0           # uniform exponent shift (cancels in softmax)
QK_SCALE = 1.0 / math.sqrt(HD)


def get_slopes(n):
    def pow2(n):
        start = 2 ** (-(2 ** (-(math.log2(n) - 3))))
        return [start * (start ** i) for i in range(n)]
    if math.log2(n).is_integer():
        return pow2(n)
    cp2 = 2 ** math.floor(math.log2(n))
    return pow2(cp2) + get_slopes(2 * cp2)[0::2][: n - cp2]


_CACHE = {}


def _build(debug=False, loop_n=0, dump=False, skip_out=False, skip_attn=False,
           skip_a=False, skip_b=False, no_recip=False):
    key = ("nc", debug, loop_n, dump, skip_out, skip_attn, skip_a, skip_b,
           no_recip)
    if key in _CACHE:
        return _CACHE[key]

    import concourse.bacc as bacc
    import concourse.tile as tile
    from concourse import mybir

    f32 = mybir.dt.float32
    bf16 = mybir.dt.bfloat16
    ACT = mybir.ActivationFunctionType

    nc = bacc.Bacc(None, target_bir_lowering=False, debug=debug)

    xT = nc.dram_tensor("xT", [C, T], bf16, kind="ExternalInput")
    wq = nc.dram_tensor("wq", [C, P], bf16, kind="ExternalInput")
    wk = nc.dram_tensor("wk", [C, P], bf16, kind="ExternalInput")
    wv = nc.dram_tensor("wv", [C, P], bf16, kind="ExternalInput")
    bq = nc.dram_tensor("bq", [P, 1], f32, kind="ExternalInput")
    bk = nc.dram_tensor("bk", [P, 1], f32, kind="ExternalInput")
    bv = nc.dram_tensor("bv", [P, 1], f32, kind="ExternalInput")
    wo = nc.dram_tensor("wo", [P, C], bf16, kind="ExternalInput")
    aliA = nc.dram_tensor("aliA", [P, W_A], f32, kind="ExternalInput")
    aliB = nc.dram_tensor("aliB", [P, NTT], f32, kind="ExternalInput")
    triA = nc.dram_tensor("triA", [P, GW], bf16, kind="ExternalInput")
    out = nc.dram_tensor("out", [T, C], bf16, kind="ExternalOutput")
    if dump:
        dqT = nc.dram_tensor("dqT", [P, T], bf16, kind="ExternalOutput")
        dkT = nc.dram_tensor("dkT", [P, T], bf16, kind="ExternalOutput")
        dvS = nc.dram_tensor("dvS", [P, NTT * 130], bf16, kind="ExternalOutput")
        doT = nc.dram_tensor("doT", [P, T], bf16, kind="ExternalOutput")

    with tile.TileContext(nc) as tc:
        with tc.tile_pool(name="consts", bufs=1) as consts, \
             tc.tile_pool(name="big", bufs=1) as big, \
             tc.tile_pool(name="xt", bufs=24) as xt_pool, \
             tc.tile_pool(name="vt", bufs=3) as vt_pool, \
             tc.tile_pool(name="etA", bufs=3) as etA_pool, \
             tc.tile_pool(name="etB", bufs=3) as etB_pool, \
             tc.tile_pool(name="lr", bufs=2) as lr_pool, \
             tc.tile_pool(name="ob", bufs=4) as ob_pool, \
             tc.tile_pool(name="ps2", bufs=2, space="PSUM") as ps2_pool, \
             tc.tile_pool(name="poA", bufs=1, space="PSUM") as poA_pool, \
             tc.tile_pool(name="poB", bufs=1, space="PSUM") as poB_pool:

            # ---- constants
            wq_sb = consts.tile([P, NCT, P], bf16)
            wk_sb = consts.tile([P, NCT, P], bf16)
            wv_sb = consts.tile([P, NCT, P], bf16)
            nc.sync.dma_start(out=wq_sb, in_=wq.ap().rearrange("(t p) d -> p t d", p=P))
            nc.sync.dma_start(out=wk_sb, in_=wk.ap().rearrange("(t p) d -> p t d", p=P))
            nc.sync.dma_start(out=wv_sb, in_=wv.ap().rearrange("(t p) d -> p t d", p=P))
            wo_sb = consts.tile([P, C], bf16)
            nc.sync.dma_start(out=wo_sb, in_=wo[:, :])
            aliA_sb = consts.tile([P, W_A], f32)
            nc.sync.dma_start(out=aliA_sb, in_=aliA[:, :])
            aliB_sb = consts.tile([P, NTT], f32)
            nc.sync.dma_start(out=aliB_sb, in_=aliB[:, :])
            triA_sb = consts.tile([P, GW], bf16)
            nc.sync.dma_start(out=triA_sb, in_=triA[:, :])
            bq_sb = consts.tile([P, 1], f32)
            bk_sb = consts.tile([P, 1], f32)
            bv_sb = consts.tile([P, 1], f32)
            nc.sync.dma_start(out=bq_sb, in_=bq[:, :])
            nc.sync.dma_start(out=bk_sb, in_=bk[:, :])
            nc.sync.dma_start(out=bv_sb, in_=bv[:, :])
            zero_bf = consts.tile([P, TBW], bf16)
            nc.vector.memset(zero_bf, 0.0)
            ident = consts.tile([P, P], bf16)
            from concourse import masks as _masks
            _masks.make_identity(nc, ident)

            # ---- persistent activations
            # partitions 0-63: head A dims; 64-127: head B dims
            qT = big.tile([P, T], bf16)
            kT = big.tile([P, T], bf16)
            # per key tile: [0:64]=v_A dims, 64=ones_A, [65:129]=v_B, 129=ones_B
            # so lhsT_A = [:, jt, 0:65] puts o_A in PSUM rows 0:64 / l_A in 64,
            # and lhsT_B = [:, jt, 65:130] puts o_B in rows 0:64 / l_B in 64.
            vS = big.tile([P, NTT, 130], bf16)
            oT = big.tile([P, T], bf16)
            for jt in range(NTT):
                nc.vector.memset(vS[:, jt, 64:65], 1.0)
                nc.vector.memset(vS[:, jt, 129:130], 1.0)

            def qkv_tblock(tb):
                cols = slice(tb * TBW, (tb + 1) * TBW)
                xts = []
                for ct in range(NCT):
                    xt = xt_pool.tile([P, TBW], bf16, tag="xt")
                    nc.sync.dma_start(
                        out=xt, in_=xT[ct * P:(ct + 1) * P, cols])
                    xts.append(xt)
                for which, w_sb, b_sb in (("q", wq_sb, bq_sb),
                                          ("k", wk_sb, bk_sb),
                                          ("v", wv_sb, bv_sb)):
                    pp = ps2_pool.tile([P, TBW], f32, tag="ps2")
                    for ct in range(NCT):
                        nc.tensor.matmul(pp, lhsT=w_sb[:, ct], rhs=xts[ct],
                                         start=(ct == 0), stop=(ct == NCT - 1))
                    if which == "q":
                        nc.vector.tensor_scalar_add(qT[:, cols], pp, b_sb)
                    elif which == "k":
                        nc.vector.tensor_scalar_add(kT[:, cols], pp, b_sb)
                    else:
                        vt = vt_pool.tile([P, TBW], bf16, tag="vt")
                        nc.scalar.activation(out=vt, in_=pp, func=ACT.Identity,
                                             bias=b_sb, scale=1.0)
                        ptr = ps2_pool.tile([P, TBW], bf16, tag="ps2")
                        for q4 in range(4):
                            nc.tensor.transpose(
                                ptr[:, q4 * P:(q4 + 1) * P],
                                vt[:, q4 * P:(q4 + 1) * P], ident)
                        for q4 in range(4):
                            jt = 4 * tb + q4
                            # strided dest: dims 0:64 -> vS[..,0:64],
                            # dims 64:128 -> vS[..,65:129]
                            dst = vS[:, jt, :].rearrange(
                                "p (a b) -> p a b", a=2)[:, :, 0:64]
                            src = ptr[:, q4 * P:(q4 + 1) * P].rearrange(
                                "p (a b) -> p a b", a=2)
                            nc.vector.tensor_copy(out=dst, in_=src)

            def body():
                for og in range(NG):
                    # ---- QKV projections for this group's two t-blocks
                    qkv_tblock(2 * og)
                    qkv_tblock(2 * og + 1)
                    if skip_attn:
                        continue

                    # ---- attention for queries q0 .. q0+1024
                    q0 = og * GW
                    its = [8 * og + g for g in range(8)]

                    poA_t = poA_pool.tile([65, GW], f32, tag="poA")
                    poB_t = poB_pool.tile([65, GW], f32, tag="poB")
                    # open the poA banks: full-width zero matmuls set
                    # has_written everywhere so the ragged-start PV matmuls
                    # below can all accumulate with start=False
                    if not skip_a:
                        nc.tensor.matmul(poA_t[:, 0:TBW], lhsT=vS[:, 0, 0:65],
                                         rhs=zero_bf, start=True, stop=False,
                                         skip_group_check=True)
                        nc.tensor.matmul(poA_t[:, TBW:GW], lhsT=vS[:, 0, 0:65],
                                         rhs=zero_bf, start=True, stop=False,
                                         skip_group_check=True)

                    a_ds = [] if skip_a else list(
                        range(min(W_A - 1, 8 * og + 7), -1, -1))
                    b_jts = [] if skip_b else list(range(8 * og + 8))

                    def _flush_a(p, last):
                        d_, gs_, et_ = p
                        for g_ in gs_:
                            jt_ = its[g_] - d_
                            nc.tensor.matmul(poA_t[:, g_ * P:(g_ + 1) * P],
                                             lhsT=vS[:, jt_, 0:65],
                                             rhs=et_[:, g_ * P:(g_ + 1) * P],
                                             start=False,
                                             stop=(last and g_ == gs_[-1]),
                                             skip_group_check=True)

                    def _flush_b(p, last):
                        jt_, col0_, et_ = p
                        for lo, hi in ([(col0_, TBW), (TBW, GW)]
                                       if col0_ < TBW else [(col0_, GW)]):
                            nc.tensor.matmul(poB_t[:, lo:hi],
                                             lhsT=vS[:, jt_, 65:130],
                                             rhs=et_[:, lo:hi],
                                             start=(jt_ == 0), stop=last,
                                             skip_group_check=True)

                    pend_a = None
                    pend_b = None
                    for step in range(max(len(a_ds), len(b_jts))):
                        # emit both S matmuls adjacently: slot B uses PE rows
                        # 64-127, slot A rows 0-63, so the array runs them
                        # concurrently via row tiling
                        jt = dj = col0 = psB_t = None
                        d = gs = g0 = psA_t = None
                        if step < len(b_jts):
                            jt = b_jts[step]
                            dj = jt - 8 * og
                            col0 = 0 if dj < 0 else P * dj
                            psB_t = ps2_pool.tile([P, GW], f32, tag="ps2")
                            for lo, hi in ([(col0, TBW), (TBW, GW)]
                                           if col0 < TBW else [(col0, GW)]):
                                nc.tensor.matmul(
                                    psB_t[:, lo:hi],
                                    lhsT=kT[64:128, jt * P:(jt + 1) * P],
                                    rhs=qT[64:128, q0 + lo:q0 + hi],
                                    start=True, stop=True)
                        if step < len(a_ds):
                            d = a_ds[step]
                            gs = [g for g in range(8) if its[g] >= d]
                            g0 = min(gs)
                            psA_t = ps2_pool.tile([P, GW], f32, tag="ps2")
                            for g in gs:
                                nc.tensor.matmul(
                                    psA_t[:, g * P:(g + 1) * P],
                                    lhsT=kT[0:64, (its[g] - d) * P:(its[g] - d + 1) * P],
                                    rhs=qT[0:64, q0 + g * P:q0 + (g + 1) * P],
                                    start=True, stop=True)
                        if psB_t is not None:
                            etB_t = etB_pool.tile([P, GW], bf16, tag="etB")
                            nc.scalar.activation(
                                out=etB_t[:, col0:GW], in_=psB_t[:, col0:GW],
                                func=ACT.Exp,
                                bias=aliB_sb[:, 8 * og - jt + 7:8 * og - jt + 8],
                                scale=QK_SCALE)
                            if dj >= 0:
                                nc.vector.tensor_mul(
                                    etB_t[:, col0:col0 + P],
                                    etB_t[:, col0:col0 + P], triA_sb[:, 0:P])
                            if pend_b is not None:
                                _flush_b(pend_b, False)
                            pend_b = (jt, col0, etB_t)
                        if psA_t is not None:
                            etA_t = etA_pool.tile([P, GW], bf16, tag="etA")
                            nc.scalar.activation(
                                out=etA_t[:, g0 * P:GW], in_=psA_t[:, g0 * P:GW],
                                func=ACT.Exp, bias=aliA_sb[:, d:d + 1],
                                scale=QK_SCALE)
                            if d == 0:
                                nc.vector.tensor_mul(etA_t[:, g0 * P:GW],
                                                     etA_t[:, g0 * P:GW],
                                                     triA_sb[:, g0 * P:GW])
                            if pend_a is not None:
                                _flush_a(pend_a, False)
                            pend_a = (d, gs, etA_t)

                    if pend_a is not None:
                        _flush_a(pend_a, True)
                    if pend_b is not None:
                        _flush_b(pend_b, True)

                    # --- bounce [O^T | l] to SBUF so the PSUM accumulator
                    #     banks free fast, then rescale O^T by 1/l from SBUF
                    #     (overlaps the next group's attention steps)
                    if not skip_a:
                        ocA = lr_pool.tile([65, GW], f32, tag="ocA")
                        nc.vector.tensor_copy(out=ocA, in_=poA_t)
                    if not skip_b:
                        ocB = lr_pool.tile([65, GW], f32, tag="ocB")
                        nc.vector.tensor_copy(out=ocB, in_=poB_t)
                    if not skip_a:
                        lrA = lr_pool.tile([1, GW], f32, tag="lrA")
                        if no_recip:
                            nc.vector.tensor_copy(out=lrA, in_=ocA[64:65, :])
                        else:
                            nc.vector.reciprocal(lrA, ocA[64:65, :])
                        lbA = lr_pool.tile([64, GW], f32, tag="lbA")
                        nc.gpsimd.partition_broadcast(lbA, lrA)
                        nc.vector.tensor_mul(oT[0:64, q0:q0 + GW],
                                             ocA[0:64, :], lbA)
                    if not skip_b:
                        lrB = lr_pool.tile([1, GW], f32, tag="lrB")
                        if no_recip:
                            nc.vector.tensor_copy(out=lrB, in_=ocB[64:65, :])
                        else:
                            nc.vector.reciprocal(lrB, ocB[64:65, :])
                        lbB = lr_pool.tile([64, GW], f32, tag="lbB")
                        nc.gpsimd.partition_broadcast(lbB, lrB)
                        nc.vector.tensor_mul(oT[64:128, q0:q0 + GW],
                                             ocB[0:64, :], lbB)

                if dump:
                    nc.sync.dma_start(out=dqT[:, :], in_=qT)
                    nc.sync.dma_start(out=dkT[:, :], in_=kT)
                    nc.sync.dma_start(out=dvS[:, :], in_=vS.rearrange("p a b -> p (a b)"))
                    nc.sync.dma_start(out=doT[:, :], in_=oT)

                # ---- output projection (dense final phase)
                for it in range(NTT if not skip_out else 0):
                    for eh in range(2):
                        ppo = ps2_pool.tile([P, TBW], f32, tag="ps2")
                        nc.tensor.matmul(ppo, lhsT=oT[:, it * P:(it + 1) * P],
                                         rhs=wo_sb[:, eh * TBW:(eh + 1) * TBW],
                                         start=True, stop=True)
                        ob = ob_pool.tile([P, TBW], bf16, tag="ob")
                        if eh % 2 == 0:
                            nc.vector.tensor_copy(out=ob, in_=ppo)
                        else:
                            nc.scalar.activation(out=ob, in_=ppo,
                                                 func=ACT.Identity, scale=1.0)
                        nc.sync.dma_start(
                            out=out[it * P:(it + 1) * P, eh * TBW:(eh + 1) * TBW],
                            in_=ob)

            # ---- optional timing loop wrapper
            import contextlib
            loop_ctx = tc.For_i(0, loop_n, 1) if loop_n else contextlib.nullcontext()
            with loop_ctx:
                body()

    nc.compile()
    _CACHE[key] = nc
    return nc


def shard_inputs(x, Wq, bq, Wk, bk, Wv, bv, Wo, bo):
    """Build the 8 per-core input maps."""
    import ml_dtypes
    x = np.asarray(x, dtype=np.float32)
    xT = np.ascontiguousarray(x.reshape(T, C).T).astype(ml_dtypes.bfloat16)
    slopes = get_slopes(H)
    jj = np.arange(P, dtype=np.float32)[:, None]          # partition index
    # 0/1 mask: valid iff query ii >= key jj, tiled GW/P wide
    tri01 = (jj <= jj.T).astype(np.float32)
    triA_np = np.tile(tri01, (1, GW // P)).astype(ml_dtypes.bfloat16)

    def col_slice(W, c):
        return np.ascontiguousarray(np.concatenate(
            [W[:, 64 * c:64 * c + 64], W[:, 64 * (8 + c):64 * (8 + c) + 64]],
            axis=1)).astype(ml_dtypes.bfloat16)

    def vec_slice(b, c):
        return np.ascontiguousarray(np.concatenate(
            [b[64 * c:64 * c + 64], b[64 * (8 + c):64 * (8 + c) + 64]])).reshape(P, 1)

    in_maps = []
    for c in range(NCORES):
        sA = np.float32(slopes[c])
        sB = np.float32(slopes[8 + c])
        dA = np.arange(W_A, dtype=np.float32)[None, :]
        aliA_np = (-sA * (128.0 * dA + 127.0 - jj) + SHIFT).astype(np.float32)
        # e = 8*og - jt in [-7, 24] -> column e + 7
        eB = np.arange(-7, NTT - 7, dtype=np.float32)[None, :]
        aliB_np = (-sB * (128.0 * eB + float(GW - 1) - jj) + SHIFT).astype(np.float32)
        in_maps.append({
            "xT": xT,
            "wq": col_slice(np.asarray(Wq, np.float32), c),
            "wk": col_slice(np.asarray(Wk, np.float32), c),
            "wv": col_slice(np.asarray(Wv, np.float32), c),
            "bq": vec_slice(np.asarray(bq, np.float32), c),
            "bk": vec_slice(np.asarray(bk, np.float32), c),
            "bv": vec_slice(np.asarray(bv, np.float32), c),
            "wo": np.ascontiguousarray(np.concatenate(
                [np.asarray(Wo, np.float32)[64 * c:64 * c + 64, :],
                 np.asarray(Wo, np.float32)[64 * (8 + c):64 * (8 + c) + 64, :]],
                axis=0)).astype(ml_dtypes.bfloat16),
            "aliA": aliA_np,
            "aliB": aliB_np,
            "triA": triA_np,
        })
    return in_maps


LAST_RESULT = None


def kernel(x, Wq, bq, Wk, bk, Wv, bv, Wo, bo, **run_kwargs):
    global LAST_RESULT
    from concourse.bass_utils import run_bass_kernel_spmd

    nc = _build()
    in_maps = shard_inputs(x, Wq, bq, Wk, bk, Wv, bv, Wo, bo)
    res = run_bass_kernel_spmd(nc, in_maps, core_ids=list(range(NCORES)), **run_kwargs)
    LAST_RESULT = res
    total = np.zeros((T, C), dtype=np.float32)
    for r in res.results:
        total += np.asarray(r["out"], dtype=np.float32)
    total += np.asarray(bo, np.float32)[None, :]
    return total.reshape(B, T, C)


# revision 24
# speedup vs baseline: 1.2554x; 1.1671x over previous
"""Causal attention with ALiBi for B=1, T=4096, C=1024, H=16 on 8 TRN2 NeuronCores.

Sharding: tensor-parallel over heads. Core c computes heads {c, 8+c}:
 - slot A = head c (steep slope): i-major over 128-query tiles, window of
   W_A=4 key tiles (keys farther than ~385 positions contribute < 1e-6 of
   softmax mass for every head in this slot). Exps batched 8 same-distance
   blocks (1024 queries) wide with a per-partition ALiBi bias column.
 - slot B = head 8+c (shallow slope): full causal window, i-major over
   1024-query windows with the ALiBi bias referenced to the window's last
   row so every live exponent stays in range; far keys underflow to 0.

All matmuls run in bf16 (S, PV, QKV, output projection). Slot A S-matmuls
(contraction 64, PE rows 0-63) and slot B S-matmuls (rows 64-127) are
emitted adjacently so the PE array runs them concurrently via row tiling.
The causal mask is applied by multiplying the bf16 exp tiles with a 0/1
lower-triangle constant on DVE. P@V matmuls carry a ones column appended
to v so PSUM accumulates [O^T | l]; O^T is rescaled by 1/l (reciprocal +
partition broadcast) directly out of PSUM into a persistent bf16 O^T
tile. The output projection runs as a dense final phase; each core
writes a bf16 [T, C] partial of out = O^T.T @ Wo_slice, and the 8
partials are summed on the host (the TP all-reduce done at unshard time)
with bo.
"""

import math

import numpy as np

B, T, C, H = 1, 4096, 1024, 16
HD = C // H            # 64
NCORES = 8
P = 128
NTT = T // P           # 32 key/query tiles
NCT = C // P           # 8 contraction tiles
TBW = 512              # t-block width for QKV projection
NTB = T // TBW         # 8
GW = 1024              # attention group width (queries)
NG = T // GW           # 4 groups
W_A = 4                # slot A window in key tiles
SHIFT = 40.0           # uniform exponent shift (cancels in softmax)
QK_SCALE = 1.0 / math.sqrt(HD)


def get_slopes(n):
    def pow2(n):
        start = 2 ** (-(2 ** (-(math.log2(n) - 3))))
        return [start * (start ** i) for i in range(n)]
    if math.log2(n).is_integer():
        return pow2(n)
    cp2 = 2 ** math.floor(math.log2(n))
    return pow2(cp2) + get_slopes(2 * cp2)[0::2][: n - cp2]


_CACHE = {}


def _build(debug=False, loop_n=0, dump=False, skip_out=False, skip_attn=False,
           skip_a=False, skip_b=False, no_recip=False):
    key = ("nc", debug, loop_n, dump, skip_out, skip_attn, skip_a, skip_b,
           no_recip)
    if key in _CACHE:
        return _CACHE[key]

    import concourse.bacc as bacc
    import concourse.tile as tile
    from concourse import mybir

    f32 = mybir.dt.float32
    bf16 = mybir.dt.bfloat16
    ACT = mybir.ActivationFunctionType

    nc = bacc.Bacc(None, target_bir_lowering=False, debug=debug)

    xT = nc.dram_tensor("xT", [C, T], bf16, kind="ExternalInput")
    wq = nc.dram_tensor("wq", [C, P], bf16, kind="ExternalInput")
    wk = nc.dram_tensor("wk", [C, P], bf16, kind="ExternalInput")
    wv = nc.dram_tensor("wv", [C, P], bf16, kind="ExternalInput")
    bq = nc.dram_tensor("bq", [P, 1], f32, kind="ExternalInput")
    bk = nc.dram_tensor("bk", [P, 1], f32, kind="ExternalInput")
    bv = nc.dram_tensor("bv", [P, 1], f32, kind="ExternalInput")
    wo = nc.dram_tensor("wo", [P, C], bf16, kind="ExternalInput")
    aliA = nc.dram_tensor("aliA", [P, W_A], f32, kind="ExternalInput")
    aliB = nc.dram_tensor("aliB", [P, NTT], f32, kind="ExternalInput")
    triA = nc.dram_tensor("triA", [P, GW], bf16, kind="ExternalInput")
    out = nc.dram_tensor("out", [T, C], bf16, kind="ExternalOutput")
    if dump:
        dqT = nc.dram_tensor("dqT", [P, T], bf16, kind="ExternalOutput")
        dkT = nc.dram_tensor("dkT", [P, T], bf16, kind="ExternalOutput")
        dvS = nc.dram_tensor("dvS", [P, NTT * 130], bf16, kind="ExternalOutput")
        doT = nc.dram_tensor("doT", [P, T], bf16, kind="ExternalOutput")

    with tile.TileContext(nc) as tc:
        with tc.tile_pool(name="consts", bufs=1) as consts, \
             tc.tile_pool(name="big", bufs=1) as big, \
             tc.tile_pool(name="xt", bufs=24) as xt_pool, \
             tc.tile_pool(name="vt", bufs=3) as vt_pool, \
             tc.tile_pool(name="etA", bufs=3) as etA_pool, \
             tc.tile_pool(name="etB", bufs=3) as etB_pool, \
             tc.tile_pool(name="lr", bufs=2) as lr_pool, \
             tc.tile_pool(name="ob", bufs=4) as ob_pool, \
             tc.tile_pool(name="ps2", bufs=2, space="PSUM") as ps2_pool, \
             tc.tile_pool(name="poA", bufs=1, space="PSUM") as poA_pool, \
             tc.tile_pool(name="poB", bufs=1, space="PSUM") as poB_pool:

            # ---- constants
            wq_sb = consts.tile([P, NCT, P], bf16)
            wk_sb = consts.tile([P, NCT, P], bf16)
            wv_sb = consts.tile([P, NCT, P], bf16)
            nc.sync.dma_start(out=wq_sb, in_=wq.ap().rearrange("(t p) d -> p t d", p=P))
            nc.sync.dma_start(out=wk_sb, in_=wk.ap().rearrange("(t p) d -> p t d", p=P))
            nc.sync.dma_start(out=wv_sb, in_=wv.ap().rearrange("(t p) d -> p t d", p=P))
            wo_sb = consts.tile([P, C], bf16)
            nc.sync.dma_start(out=wo_sb, in_=wo[:, :])
            aliA_sb = consts.tile([P, W_A], f32)
            nc.sync.dma_start(out=aliA_sb, in_=aliA[:, :])
            aliB_sb = consts.tile([P, NTT], f32)
            nc.sync.dma_start(out=aliB_sb, in_=aliB[:, :])
            triA_sb = consts.tile([P, GW], bf16)
            nc.sync.dma_start(out=triA_sb, in_=triA[:, :])
            bq_sb = consts.tile([P, 1], f32)
            bk_sb = consts.tile([P, 1], f32)
            bv_sb = consts.tile([P, 1], f32)
            nc.sync.dma_start(out=bq_sb, in_=bq[:, :])
            nc.sync.dma_start(out=bk_sb, in_=bk[:, :])
            nc.sync.dma_start(out=bv_sb, in_=bv[:, :])
            zero_bf = consts.tile([P, TBW], bf16)
            nc.vector.memset(zero_bf, 0.0)
            ident = consts.tile([P, P], bf16)
            from concourse import masks as _masks
            _masks.make_identity(nc, ident)

            # ---- persistent activations
            # partitions 0-63: head A dims; 64-127: head B dims
            qT = big.tile([P, T], bf16)
            kT = big.tile([P, T], bf16)
            # per key tile: [0:64]=v_A dims, 64=ones_A, [65:129]=v_B, 129=ones_B
            # so lhsT_A = [:, jt, 0:65] puts o_A in PSUM rows 0:64 / l_A in 64,
            # and lhsT_B = [:, jt, 65:130] puts o_B in rows 0:64 / l_B in 64.
            vS = big.tile([P, NTT, 130], bf16)
            oT = big.tile([P, T], bf16)
            for jt in range(NTT):
                nc.vector.memset(vS[:, jt, 64:65], 1.0)
                nc.vector.memset(vS[:, jt, 129:130], 1.0)

            def qkv_tblock(tb):
                cols = slice(tb * TBW, (tb + 1) * TBW)
                xts = []
                for ct in range(NCT):
                    xt = xt_pool.tile([P, TBW], bf16, tag="xt")
                    nc.sync.dma_start(
                        out=xt, in_=xT[ct * P:(ct + 1) * P, cols])
                    xts.append(xt)
                # q and k psums pack into the two banks of one pool slot
                pqk = ps2_pool.tile([P, GW], f32, tag="ps2")
                for half, w_sb in ((0, wq_sb), (1, wk_sb)):
                    for ct in range(NCT):
                        nc.tensor.matmul(pqk[:, half * TBW:(half + 1) * TBW],
                                         lhsT=w_sb[:, ct], rhs=xts[ct],
                                         start=(ct == 0), stop=(ct == NCT - 1))
                nc.vector.tensor_scalar_add(qT[:, cols], pqk[:, 0:TBW], bq_sb)
                nc.vector.tensor_scalar_add(kT[:, cols], pqk[:, TBW:GW], bk_sb)
                # v psum in bank 0 of a second slot; transposes write bf16
                # into a bitcast view of bank 1
                pvt = ps2_pool.tile([P, GW], f32, tag="ps2")
                for ct in range(NCT):
                    nc.tensor.matmul(pvt[:, 0:TBW], lhsT=wv_sb[:, ct],
                                     rhs=xts[ct],
                                     start=(ct == 0), stop=(ct == NCT - 1))
                vt = vt_pool.tile([P, TBW], bf16, tag="vt")
                nc.scalar.activation(out=vt, in_=pvt[:, 0:TBW],
                                     func=ACT.Identity, bias=bv_sb, scale=1.0)
                ptr = pvt[:, TBW:GW].bitcast(bf16)
                for q4 in range(4):
                    nc.tensor.transpose(
                        ptr[:, q4 * P:(q4 + 1) * P],
                        vt[:, q4 * P:(q4 + 1) * P], ident)
                for q4 in range(4):
                    jt = 4 * tb + q4
                    # strided dest: dims 0:64 -> vS[..,0:64],
                    # dims 64:128 -> vS[..,65:129]
                    dst = vS[:, jt, :].rearrange(
                        "p (a b) -> p a b", a=2)[:, :, 0:64]
                    src = ptr[:, q4 * P:(q4 + 1) * P].rearrange(
                        "p (a b) -> p a b", a=2)
                    nc.vector.tensor_copy(out=dst, in_=src)

            def body():
                for og in range(NG):
                    # ---- QKV projections for this group's two t-blocks
                    qkv_tblock(2 * og)
                    qkv_tblock(2 * og + 1)
                    if skip_attn:
                        continue

                    # ---- attention for queries q0 .. q0+1024
                    q0 = og * GW
                    its = [8 * og + g for g in range(8)]

                    poA_t = poA_pool.tile([65, GW], f32, tag="poA")
                    poB_t = poB_pool.tile([65, GW], f32, tag="poB")
                    # open the poA banks: full-width zero matmuls set
                    # has_written everywhere so the ragged-start PV matmuls
                    # below can all accumulate with start=False
                    if not skip_a:
                        nc.tensor.matmul(poA_t[:, 0:TBW], lhsT=vS[:, 0, 0:65],
                                         rhs=zero_bf, start=True, stop=False,
                                         skip_group_check=True)
                        nc.tensor.matmul(poA_t[:, TBW:GW], lhsT=vS[:, 0, 0:65],
                                         rhs=zero_bf, start=True, stop=False,
                                         skip_group_check=True)

                    a_ds = [] if skip_a else list(
                        range(min(W_A - 1, 8 * og + 7), -1, -1))
                    b_jts = [] if skip_b else list(range(8 * og + 8))

                    def _flush_a(p, last):
                        d_, gs_, et_ = p
                        for g_ in gs_:
                            jt_ = its[g_] - d_
                            nc.tensor.matmul(poA_t[:, g_ * P:(g_ + 1) * P],
                                             lhsT=vS[:, jt_, 0:65],
                                             rhs=et_[:, g_ * P:(g_ + 1) * P],
                                             start=False,
                                             stop=(last and g_ == gs_[-1]),
                                             skip_group_check=True)

                    def _flush_b(p, last):
                        jt_, col0_, et_ = p
                        for lo, hi in ([(col0_, TBW), (TBW, GW)]
                                       if col0_ < TBW else [(col0_, GW)]):
                            nc.tensor.matmul(poB_t[:, lo:hi],
                                             lhsT=vS[:, jt_, 65:130],
                                             rhs=et_[:, lo:hi],
                                             start=(jt_ == 0), stop=last,
                                             skip_group_check=True)

                    pend_a = None
                    pend_b = None
                    for step in range(max(len(a_ds), len(b_jts))):
                        # emit both S matmuls adjacently: slot B uses PE rows
                        # 64-127, slot A rows 0-63, so the array runs them
                        # concurrently via row tiling
                        jt = dj = col0 = psB_t = None
                        d = gs = g0 = psA_t = None
                        if step < len(b_jts):
                            jt = b_jts[step]
                            dj = jt - 8 * og
                            col0 = 0 if dj < 0 else P * dj
                            psB_t = ps2_pool.tile([P, GW], f32, tag="ps2")
                            for lo, hi in ([(col0, TBW), (TBW, GW)]
                                           if col0 < TBW else [(col0, GW)]):
                                nc.tensor.matmul(
                                    psB_t[:, lo:hi],
                                    lhsT=kT[64:128, jt * P:(jt + 1) * P],
                                    rhs=qT[64:128, q0 + lo:q0 + hi],
                                    start=True, stop=True)
                        if step < len(a_ds):
                            d = a_ds[step]
                            gs = [g for g in range(8) if its[g] >= d]
                            g0 = min(gs)
                            psA_t = ps2_pool.tile([P, GW], f32, tag="ps2")
                            for g in gs:
                                nc.tensor.matmul(
                                    psA_t[:, g * P:(g + 1) * P],
                                    lhsT=kT[0:64, (its[g] - d) * P:(its[g] - d + 1) * P],
                                    rhs=qT[0:64, q0 + g * P:q0 + (g + 1) * P],
                                    start=True, stop=True)
                        if psB_t is not None:
                            etB_t = etB_pool.tile([P, GW], bf16, tag="etB")
                            nc.scalar.activation(
                                out=etB_t[:, col0:GW], in_=psB_t[:, col0:GW],
                                func=ACT.Exp,
                                bias=aliB_sb[:, 8 * og - jt + 7:8 * og - jt + 8],
                                scale=QK_SCALE)
                            if dj >= 0:
                                nc.vector.tensor_mul(
                                    etB_t[:, col0:col0 + P],
                                    etB_t[:, col0:col0 + P], triA_sb[:, 0:P])
                            if pend_b is not None:
                                _flush_b(pend_b, False)
                            pend_b = (jt, col0, etB_t)
                        if psA_t is not None:
                            etA_t = etA_pool.tile([P, GW], bf16, tag="etA")
                            nc.scalar.activation(
                                out=etA_t[:, g0 * P:GW], in_=psA_t[:, g0 * P:GW],
                                func=ACT.Exp, bias=aliA_sb[:, d:d + 1],
                                scale=QK_SCALE)
                            if d == 0:
                                nc.vector.tensor_mul(etA_t[:, g0 * P:GW],
                                                     etA_t[:, g0 * P:GW],
                                                     triA_sb[:, g0 * P:GW])
                            if pend_a is not None:
                                _flush_a(pend_a, False)
                            pend_a = (d, gs, etA_t)

                    if pend_a is not None:
                        _flush_a(pend_a, True)
                    if pend_b is not None:
                        _flush_b(pend_b, True)

                    # --- bounce [O^T | l] to SBUF so the PSUM accumulator
                    #     banks free fast, then rescale O^T by 1/l from SBUF
                    #     (overlaps the next group's attention steps)
                    if not skip_a:
                        ocA = lr_pool.tile([65, GW], f32, tag="ocA")
                        nc.vector.tensor_copy(out=ocA, in_=poA_t)
                    if not skip_b:
                        ocB = lr_pool.tile([65, GW], f32, tag="ocB")
                        nc.vector.tensor_copy(out=ocB, in_=poB_t)
                    if not skip_a:
                        lrA = lr_pool.tile([1, GW], f32, tag="lrA")
                        if no_recip:
                            nc.vector.tensor_copy(out=lrA, in_=ocA[64:65, :])
                        else:
                            nc.vector.reciprocal(lrA, ocA[64:65, :])
                        lbA = lr_pool.tile([64, GW], f32, tag="lbA")
                        nc.gpsimd.partition_broadcast(lbA, lrA)
                        nc.vector.tensor_mul(oT[0:64, q0:q0 + GW],
                                             ocA[0:64, :], lbA)
                    if not skip_b:
                        lrB = lr_pool.tile([1, GW], f32, tag="lrB")
                        if no_recip:
                            nc.vector.tensor_copy(out=lrB, in_=ocB[64:65, :])
                        else:
                            nc.vector.reciprocal(lrB, ocB[64:65, :])
                        lbB = lr_pool.tile([64, GW], f32, tag="lbB")
                        nc.gpsimd.partition_broadcast(lbB, lrB)
                        nc.vector.tensor_mul(oT[64:128, q0:q0 + GW],
                                             ocB[0:64, :], lbB)

                if dump:
                    nc.sync.dma_start(out=dqT[:, :], in_=qT)
                    nc.sync.dma_start(out=dkT[:, :], in_=kT)
                    nc.sync.dma_start(out=dvS[:, :], in_=vS.rearrange("p a b -> p (a b)"))
                    nc.sync.dma_start(out=doT[:, :], in_=oT)

                # ---- output projection (dense final phase)
                for it in range(NTT if not skip_out else 0):
                    ppo = ps2_pool.tile([P, GW], f32, tag="ps2")
                    for eh in range(2):
                        nc.tensor.matmul(ppo[:, eh * TBW:(eh + 1) * TBW],
                                         lhsT=oT[:, it * P:(it + 1) * P],
                                         rhs=wo_sb[:, eh * TBW:(eh + 1) * TBW],
                                         start=True, stop=True)
                    ob = ob_pool.tile([P, GW], bf16, tag="ob")
                    nc.vector.tensor_copy(out=ob[:, 0:TBW], in_=ppo[:, 0:TBW])
                    nc.scalar.activation(out=ob[:, TBW:GW], in_=ppo[:, TBW:GW],
                                         func=ACT.Identity, scale=1.0)
                    nc.sync.dma_start(
                        out=out[it * P:(it + 1) * P, :], in_=ob)

            # ---- optional timing loop wrapper
            import contextlib
            loop_ctx = tc.For_i(0, loop_n, 1) if loop_n else contextlib.nullcontext()
            with loop_ctx:
                body()

    nc.compile()
    _CACHE[key] = nc
    return nc


def shard_inputs(x, Wq, bq, Wk, bk, Wv, bv, Wo, bo):
    """Build the 8 per-core input maps."""
    import ml_dtypes
    x = np.asarray(x, dtype=np.float32)
    xT = np.ascontiguousarray(x.reshape(T, C).T).astype(ml_dtypes.bfloat16)
    slopes = get_slopes(H)
    jj = np.arange(P, dtype=np.float32)[:, None]          # partition index
    # 0/1 mask: valid iff query ii >= key jj, tiled GW/P wide
    tri01 = (jj <= jj.T).astype(np.float32)
    triA_np = np.tile(tri01, (1, GW // P)).astype(ml_dtypes.bfloat16)

    def col_slice(W, c):
        return np.ascontiguousarray(np.concatenate(
            [W[:, 64 * c:64 * c + 64], W[:, 64 * (8 + c):64 * (8 + c) + 64]],
            axis=1)).astype(ml_dtypes.bfloat16)

    def vec_slice(b, c):
        return np.ascontiguousarray(np.concatenate(
            [b[64 * c:64 * c + 64], b[64 * (8 + c):64 * (8 + c) + 64]])).reshape(P, 1)

    in_maps = []
    for c in range(NCORES):
        sA = np.float32(slopes[c])
        sB = np.float32(slopes[8 + c])
        dA = np.arange(W_A, dtype=np.float32)[None, :]
        aliA_np = (-sA * (128.0 * dA + 127.0 - jj) + SHIFT).astype(np.float32)
        # e = 8*og - jt in [-7, 24] -> column e + 7
        eB = np.arange(-7, NTT - 7, dtype=np.float32)[None, :]
        aliB_np = (-sB * (128.0 * eB + float(GW - 1) - jj) + SHIFT).astype(np.float32)
        in_maps.append({
            "xT": xT,
            "wq": col_slice(np.asarray(Wq, np.float32), c),
            "wk": col_slice(np.asarray(Wk, np.float32), c),
            "wv": col_slice(np.asarray(Wv, np.float32), c),
            "bq": vec_slice(np.asarray(bq, np.float32), c),
            "bk": vec_slice(np.asarray(bk, np.float32), c),
            "bv": vec_slice(np.asarray(bv, np.float32), c),
            "wo": np.ascontiguousarray(np.concatenate(
                [np.asarray(Wo, np.float32)[64 * c:64 * c + 64, :],
                 np.asarray(Wo, np.float32)[64 * (8 + c):64 * (8 + c) + 64, :]],
                axis=0)).astype(ml_dtypes.bfloat16),
            "aliA": aliA_np,
            "aliB": aliB_np,
            "triA": triA_np,
        })
    return in_maps


LAST_RESULT = None


def kernel(x, Wq, bq, Wk, bk, Wv, bv, Wo, bo, **run_kwargs):
    global LAST_RESULT
    from concourse.bass_utils import run_bass_kernel_spmd

    nc = _build()
    in_maps = shard_inputs(x, Wq, bq, Wk, bk, Wv, bv, Wo, bo)
    res = run_bass_kernel_spmd(nc, in_maps, core_ids=list(range(NCORES)), **run_kwargs)
    LAST_RESULT = res
    total = np.zeros((T, C), dtype=np.float32)
    for r in res.results:
        total += np.asarray(r["out"], dtype=np.float32)
    total += np.asarray(bo, np.float32)[None, :]
    return total.reshape(B, T, C)


# revision 27
# speedup vs baseline: 1.3742x; 1.0946x over previous
"""Causal attention with ALiBi for B=1, T=4096, C=1024, H=16 on 8 TRN2 NeuronCores.

Sharding: tensor-parallel over heads. Core c computes heads {c, 8+c}:
 - slot A = head c (steep slope): i-major over 128-query tiles, window of
   W_A=4 key tiles (keys farther than ~385 positions contribute < 1e-6 of
   softmax mass for every head in this slot). Exps batched 8 same-distance
   blocks (1024 queries) wide with a per-partition ALiBi bias column.
 - slot B = head 8+c (shallow slope): full causal window, i-major over
   1024-query windows with the ALiBi bias referenced to the window's last
   row so every live exponent stays in range; far keys underflow to 0.

All matmuls run in bf16 (S, PV, QKV, output projection). Slot A S-matmuls
(contraction 64, PE rows 0-63) and slot B S-matmuls (rows 64-127) are
emitted adjacently so the PE array runs them concurrently via row tiling.
The causal mask is applied by multiplying the bf16 exp tiles with a 0/1
lower-triangle constant on DVE. P@V matmuls carry a ones column appended
to v so PSUM accumulates [O^T | l]; O^T is rescaled by 1/l (reciprocal +
partition broadcast) directly out of PSUM into a persistent bf16 O^T
tile. The output projection runs as a dense final phase; each core
writes a bf16 [T, C] partial of out = O^T.T @ Wo_slice, and the 8
partials are summed on the host (the TP all-reduce done at unshard time)
with bo.
"""

import math

import numpy as np

B, T, C, H = 1, 4096, 1024, 16
HD = C // H            # 64
NCORES = 8
P = 128
NTT = T // P           # 32 key/query tiles
NCT = C // P           # 8 contraction tiles
TBW = 512              # t-block width for QKV projection
NTB = T // TBW         # 8
GW = 1024              # attention group width (queries)
NG = T // GW           # 4 groups
W_A = 4                # slot A window in key tiles
SHIFT = 40.0           # uniform exponent shift (cancels in softmax)
QK_SCALE = 1.0 / math.sqrt(HD)


def get_slopes(n):
    def pow2(n):
        start = 2 ** (-(2 ** (-(math.log2(n) - 3))))
        return [start * (start ** i) for i in range(n)]
    if math.log2(n).is_integer():
        return pow2(n)
    cp2 = 2 ** math.floor(math.log2(n))
    return pow2(cp2) + get_slopes(2 * cp2)[0::2][: n - cp2]


_CACHE = {}


def _build(debug=False, loop_n=0, dump=False, skip_out=False, skip_attn=False,
           skip_a=False, skip_b=False, no_recip=False):
    key = ("nc", debug, loop_n, dump, skip_out, skip_attn, skip_a, skip_b,
           no_recip)
    if key in _CACHE:
        return _CACHE[key]

    import concourse.bacc as bacc
    import concourse.tile as tile
    from concourse import mybir

    f32 = mybir.dt.float32
    bf16 = mybir.dt.bfloat16
    ACT = mybir.ActivationFunctionType

    nc = bacc.Bacc(None, target_bir_lowering=False, debug=debug)

    xT = nc.dram_tensor("xT", [C, T], bf16, kind="ExternalInput")
    wq = nc.dram_tensor("wq", [C, P], bf16, kind="ExternalInput")
    wk = nc.dram_tensor("wk", [C, P], bf16, kind="ExternalInput")
    wv = nc.dram_tensor("wv", [C, P], bf16, kind="ExternalInput")
    bq = nc.dram_tensor("bq", [P, 1], f32, kind="ExternalInput")
    bk = nc.dram_tensor("bk", [P, 1], f32, kind="ExternalInput")
    bv = nc.dram_tensor("bv", [P, 1], f32, kind="ExternalInput")
    wo = nc.dram_tensor("wo", [P, C], bf16, kind="ExternalInput")
    aliA = nc.dram_tensor("aliA", [P, W_A], f32, kind="ExternalInput")
    aliB = nc.dram_tensor("aliB", [P, NTT], f32, kind="ExternalInput")
    triA = nc.dram_tensor("triA", [P, GW], bf16, kind="ExternalInput")
    out = nc.dram_tensor("out", [T, C], bf16, kind="ExternalOutput")
    if dump:
        dqT = nc.dram_tensor("dqT", [P, T], bf16, kind="ExternalOutput")
        dkT = nc.dram_tensor("dkT", [P, T], bf16, kind="ExternalOutput")
        dvS = nc.dram_tensor("dvS", [P, NTT * 130], bf16, kind="ExternalOutput")
        doT = nc.dram_tensor("doT", [P, T], bf16, kind="ExternalOutput")

    with tile.TileContext(nc) as tc:
        with tc.tile_pool(name="consts", bufs=1) as consts, \
             tc.tile_pool(name="big", bufs=1) as big, \
             tc.tile_pool(name="xt", bufs=24) as xt_pool, \
             tc.tile_pool(name="vt", bufs=3) as vt_pool, \
             tc.tile_pool(name="etA", bufs=3) as etA_pool, \
             tc.tile_pool(name="etB", bufs=3) as etB_pool, \
             tc.tile_pool(name="lr", bufs=2) as lr_pool, \
             tc.tile_pool(name="ob", bufs=4) as ob_pool, \
             tc.tile_pool(name="ps2", bufs=2, space="PSUM") as ps2_pool, \
             tc.tile_pool(name="poA", bufs=1, space="PSUM") as poA_pool, \
             tc.tile_pool(name="poB", bufs=1, space="PSUM") as poB_pool:

            # ---- constants
            wq_sb = consts.tile([P, NCT, P], bf16)
            wk_sb = consts.tile([P, NCT, P], bf16)
            wv_sb = consts.tile([P, NCT, P], bf16)
            nc.sync.dma_start(out=wq_sb, in_=wq.ap().rearrange("(t p) d -> p t d", p=P))
            nc.sync.dma_start(out=wk_sb, in_=wk.ap().rearrange("(t p) d -> p t d", p=P))
            nc.sync.dma_start(out=wv_sb, in_=wv.ap().rearrange("(t p) d -> p t d", p=P))
            wo_sb = consts.tile([P, C], bf16)
            nc.sync.dma_start(out=wo_sb, in_=wo[:, :])
            aliA_sb = consts.tile([P, W_A], f32)
            nc.sync.dma_start(out=aliA_sb, in_=aliA[:, :])
            aliB_sb = consts.tile([P, NTT], f32)
            nc.sync.dma_start(out=aliB_sb, in_=aliB[:, :])
            triA_sb = consts.tile([P, GW], bf16)
            nc.sync.dma_start(out=triA_sb, in_=triA[:, :])
            bq_sb = consts.tile([P, 1], f32)
            bk_sb = consts.tile([P, 1], f32)
            bv_sb = consts.tile([P, 1], f32)
            nc.sync.dma_start(out=bq_sb, in_=bq[:, :])
            nc.sync.dma_start(out=bk_sb, in_=bk[:, :])
            nc.sync.dma_start(out=bv_sb, in_=bv[:, :])
            zero_bf = consts.tile([P, TBW], bf16)
            nc.vector.memset(zero_bf, 0.0)
            ident = consts.tile([P, P], bf16)
            from concourse import masks as _masks
            _masks.make_identity(nc, ident)

            # ---- persistent activations
            # partitions 0-63: head A dims; 64-127: head B dims
            qT = big.tile([P, T], bf16)
            kT = big.tile([P, T], bf16)
            # per key tile: [0:64]=v_A dims, 64=ones_A, [65:129]=v_B, 129=ones_B
            # so lhsT_A = [:, jt, 0:65] puts o_A in PSUM rows 0:64 / l_A in 64,
            # and lhsT_B = [:, jt, 65:130] puts o_B in rows 0:64 / l_B in 64.
            vS = big.tile([P, NTT, 130], bf16)
            oT = big.tile([P, T], bf16)
            ltA = big.tile([32, GW], f32)
            lttA = big.tile([32, GW], f32)
            ltB = big.tile([32, GW], f32)
            lttB = big.tile([32, GW], f32)
            nc.vector.memset(ltA, 1.0)
            nc.vector.memset(lttA, 1.0)
            nc.vector.memset(ltB, 1.0)
            nc.vector.memset(lttB, 1.0)
            for jt in range(NTT):
                nc.vector.memset(vS[:, jt, 64:65], 1.0)
                nc.vector.memset(vS[:, jt, 129:130], 1.0)

            def qkv_tblock(tb):
                cols = slice(tb * TBW, (tb + 1) * TBW)
                xts = []
                for ct in range(NCT):
                    xt = xt_pool.tile([P, TBW], bf16, tag="xt")
                    nc.sync.dma_start(
                        out=xt, in_=xT[ct * P:(ct + 1) * P, cols])
                    xts.append(xt)
                # q and k psums pack into the two banks of one pool slot
                pqk = ps2_pool.tile([P, GW], f32, tag="ps2")
                for half, w_sb in ((0, wq_sb), (1, wk_sb)):
                    for ct in range(NCT):
                        nc.tensor.matmul(pqk[:, half * TBW:(half + 1) * TBW],
                                         lhsT=w_sb[:, ct], rhs=xts[ct],
                                         start=(ct == 0), stop=(ct == NCT - 1))
                nc.vector.tensor_scalar_add(qT[:, cols], pqk[:, 0:TBW], bq_sb)
                nc.vector.tensor_scalar_add(kT[:, cols], pqk[:, TBW:GW], bk_sb)
                # v psum in bank 0 of a second slot; transposes write bf16
                # into a bitcast view of bank 1
                pvt = ps2_pool.tile([P, GW], f32, tag="ps2")
                for ct in range(NCT):
                    nc.tensor.matmul(pvt[:, 0:TBW], lhsT=wv_sb[:, ct],
                                     rhs=xts[ct],
                                     start=(ct == 0), stop=(ct == NCT - 1))
                vt = vt_pool.tile([P, TBW], bf16, tag="vt")
                nc.scalar.activation(out=vt, in_=pvt[:, 0:TBW],
                                     func=ACT.Identity, bias=bv_sb, scale=1.0)
                ptr = pvt[:, TBW:GW].bitcast(bf16)
                for q4 in range(4):
                    nc.tensor.transpose(
                        ptr[:, q4 * P:(q4 + 1) * P],
                        vt[:, q4 * P:(q4 + 1) * P], ident)
                for q4 in range(4):
                    jt = 4 * tb + q4
                    # strided dest: dims 0:64 -> vS[..,0:64],
                    # dims 64:128 -> vS[..,65:129]
                    dst = vS[:, jt, :].rearrange(
                        "p (a b) -> p a b", a=2)[:, :, 0:64]
                    src = ptr[:, q4 * P:(q4 + 1) * P].rearrange(
                        "p (a b) -> p a b", a=2)
                    nc.vector.tensor_copy(out=dst, in_=src)

            def body():
                for og in range(NG):
                    # ---- QKV projections for this group's two t-blocks
                    qkv_tblock(2 * og)
                    qkv_tblock(2 * og + 1)
                    if skip_attn:
                        continue

                    # ---- attention for queries q0 .. q0+1024
                    q0 = og * GW
                    its = [8 * og + g for g in range(8)]

                    poA_t = poA_pool.tile([65, GW], f32, tag="poA")
                    poB_t = poB_pool.tile([65, GW], f32, tag="poB")
                    # open the poA banks: full-width zero matmuls set
                    # has_written everywhere so the ragged-start PV matmuls
                    # below can all accumulate with start=False
                    if not skip_a:
                        nc.tensor.matmul(poA_t[:, 0:TBW], lhsT=vS[:, 0, 0:65],
                                         rhs=zero_bf, start=True, stop=False,
                                         skip_group_check=True)
                        nc.tensor.matmul(poA_t[:, TBW:GW], lhsT=vS[:, 0, 0:65],
                                         rhs=zero_bf, start=True, stop=False,
                                         skip_group_check=True)

                    a_ds = [] if skip_a else list(
                        range(min(W_A - 1, 8 * og + 7), -1, -1))
                    b_jts = [] if skip_b else list(range(8 * og + 8))

                    def _flush_a(p, last):
                        d_, gs_, et_ = p
                        for g_ in gs_:
                            jt_ = its[g_] - d_
                            nc.tensor.matmul(poA_t[:, g_ * P:(g_ + 1) * P],
                                             lhsT=vS[:, jt_, 0:65],
                                             rhs=et_[:, g_ * P:(g_ + 1) * P],
                                             start=False,
                                             stop=(last and g_ == gs_[-1]),
                                             skip_group_check=True)

                    def _flush_b(p, last):
                        jt_, col0_, et_ = p
                        for lo, hi in ([(col0_, TBW), (TBW, GW)]
                                       if col0_ < TBW else [(col0_, GW)]):
                            nc.tensor.matmul(poB_t[:, lo:hi],
                                             lhsT=vS[:, jt_, 65:130],
                                             rhs=et_[:, lo:hi],
                                             start=(jt_ == 0), stop=last,
                                             skip_group_check=True)

                    pend_a = None
                    pend_b = None
                    for step in range(max(len(a_ds), len(b_jts))):
                        # emit both S matmuls adjacently: slot B uses PE rows
                        # 64-127, slot A rows 0-63, so the array runs them
                        # concurrently via row tiling
                        jt = dj = col0 = psB_t = None
                        d = gs = g0 = psA_t = None
                        if step < len(b_jts):
                            jt = b_jts[step]
                            dj = jt - 8 * og
                            col0 = 0 if dj < 0 else P * dj
                            psB_t = ps2_pool.tile([P, GW], f32, tag="ps2")
                            for lo, hi in ([(col0, TBW), (TBW, GW)]
                                           if col0 < TBW else [(col0, GW)]):
                                nc.tensor.matmul(
                                    psB_t[:, lo:hi],
                                    lhsT=kT[64:128, jt * P:(jt + 1) * P],
                                    rhs=qT[64:128, q0 + lo:q0 + hi],
                                    start=True, stop=True)
                        if step < len(a_ds):
                            d = a_ds[step]
                            gs = [g for g in range(8) if its[g] >= d]
                            g0 = min(gs)
                            psA_t = ps2_pool.tile([P, GW], f32, tag="ps2")
                            for g in gs:
                                nc.tensor.matmul(
                                    psA_t[:, g * P:(g + 1) * P],
                                    lhsT=kT[0:64, (its[g] - d) * P:(its[g] - d + 1) * P],
                                    rhs=qT[0:64, q0 + g * P:q0 + (g + 1) * P],
                                    start=True, stop=True)
                        if psB_t is not None:
                            etB_t = etB_pool.tile([P, GW], bf16, tag="etB")
                            nc.scalar.activation(
                                out=etB_t[:, col0:GW], in_=psB_t[:, col0:GW],
                                func=ACT.Exp,
                                bias=aliB_sb[:, 8 * og - jt + 7:8 * og - jt + 8],
                                scale=QK_SCALE)
                            if dj >= 0:
                                nc.vector.tensor_mul(
                                    etB_t[:, col0:col0 + P],
                                    etB_t[:, col0:col0 + P], triA_sb[:, 0:P])
                            if pend_b is not None:
                                _flush_b(pend_b, False)
                            pend_b = (jt, col0, etB_t)
                        if psA_t is not None:
                            etA_t = etA_pool.tile([P, GW], bf16, tag="etA")
                            nc.scalar.activation(
                                out=etA_t[:, g0 * P:GW], in_=psA_t[:, g0 * P:GW],
                                func=ACT.Exp, bias=aliA_sb[:, d:d + 1],
                                scale=QK_SCALE)
                            if d == 0:
                                nc.vector.tensor_mul(etA_t[:, g0 * P:GW],
                                                     etA_t[:, g0 * P:GW],
                                                     triA_sb[:, g0 * P:GW])
                            if pend_a is not None:
                                _flush_a(pend_a, False)
                            pend_a = (d, gs, etA_t)

                    if pend_a is not None:
                        _flush_a(pend_a, True)
                    if pend_b is not None:
                        _flush_b(pend_b, True)

                    # --- bounce [O^T | l] to SBUF so the PSUM accumulator
                    #     banks free fast, then rescale O^T by 1/l from SBUF
                    #     (overlaps the next group's attention steps)
                    if not skip_a:
                        ocA = lr_pool.tile([65, GW], f32, tag="ocA")
                        nc.vector.tensor_copy(out=ocA, in_=poA_t)
                    if not skip_b:
                        ocB = lr_pool.tile([65, GW], f32, tag="ocB")
                        nc.vector.tensor_copy(out=ocB, in_=poB_t)
                    def _recip_row(oc, lt, ltt, tag):
                        # spread the [1, GW] l-row over 32 DVE lanes via a
                        # 32x32 block transpose so the iterative-divide runs
                        # 32 elems/lane instead of GW on one lane
                        if no_recip:
                            lr = lr_pool.tile([1, GW], f32, tag="lr" + tag)
                            nc.vector.tensor_copy(out=lr, in_=oc[64:65, :])
                            return lr
                        nc.vector.tensor_copy(out=lt[0:1, :], in_=oc[64:65, :])
                        nc.vector.transpose(ltt, lt)
                        nc.vector.reciprocal(ltt[:, 0:GW:32], ltt[:, 0:GW:32])
                        nc.vector.transpose(lt, ltt)
                        return lt[0:1, :]
                    if not skip_a:
                        lrA = _recip_row(ocA, ltA, lttA, "A")
                        lbA = lr_pool.tile([64, GW], f32, tag="lbA")
                        nc.gpsimd.partition_broadcast(lbA, lrA)
                        nc.vector.tensor_mul(oT[0:64, q0:q0 + GW],
                                             ocA[0:64, :], lbA)
                    if not skip_b:
                        lrB = _recip_row(ocB, ltB, lttB, "B")
                        lbB = lr_pool.tile([64, GW], f32, tag="lbB")
                        nc.gpsimd.partition_broadcast(lbB, lrB)
                        nc.vector.tensor_mul(oT[64:128, q0:q0 + GW],
                                             ocB[0:64, :], lbB)

                if dump:
                    nc.sync.dma_start(out=dqT[:, :], in_=qT)
                    nc.sync.dma_start(out=dkT[:, :], in_=kT)
                    nc.sync.dma_start(out=dvS[:, :], in_=vS.rearrange("p a b -> p (a b)"))
                    nc.sync.dma_start(out=doT[:, :], in_=oT)

                # ---- output projection (dense final phase)
                for it in range(NTT if not skip_out else 0):
                    ppo = ps2_pool.tile([P, GW], f32, tag="ps2")
                    for eh in range(2):
                        nc.tensor.matmul(ppo[:, eh * TBW:(eh + 1) * TBW],
                                         lhsT=oT[:, it * P:(it + 1) * P],
                                         rhs=wo_sb[:, eh * TBW:(eh + 1) * TBW],
                                         start=True, stop=True)
                    ob = ob_pool.tile([P, GW], bf16, tag="ob")
                    nc.vector.tensor_copy(out=ob[:, 0:TBW], in_=ppo[:, 0:TBW])
                    nc.scalar.activation(out=ob[:, TBW:GW], in_=ppo[:, TBW:GW],
                                         func=ACT.Identity, scale=1.0)
                    nc.sync.dma_start(
                        out=out[it * P:(it + 1) * P, :], in_=ob)

            # ---- optional timing loop wrapper
            import contextlib
            loop_ctx = tc.For_i(0, loop_n, 1) if loop_n else contextlib.nullcontext()
            with loop_ctx:
                body()

    nc.compile()
    _CACHE[key] = nc
    return nc


def shard_inputs(x, Wq, bq, Wk, bk, Wv, bv, Wo, bo):
    """Build the 8 per-core input maps."""
    import ml_dtypes
    x = np.asarray(x, dtype=np.float32)
    xT = np.ascontiguousarray(x.reshape(T, C).T).astype(ml_dtypes.bfloat16)
    slopes = get_slopes(H)
    jj = np.arange(P, dtype=np.float32)[:, None]          # partition index
    # 0/1 mask: valid iff query ii >= key jj, tiled GW/P wide
    tri01 = (jj <= jj.T).astype(np.float32)
    triA_np = np.tile(tri01, (1, GW // P)).astype(ml_dtypes.bfloat16)

    def col_slice(W, c):
        return np.ascontiguousarray(np.concatenate(
            [W[:, 64 * c:64 * c + 64], W[:, 64 * (8 + c):64 * (8 + c) + 64]],
            axis=1)).astype(ml_dtypes.bfloat16)

    def vec_slice(b, c):
        return np.ascontiguousarray(np.concatenate(
            [b[64 * c:64 * c + 64], b[64 * (8 + c):64 * (8 + c) + 64]])).reshape(P, 1)

    in_maps = []
    for c in range(NCORES):
        sA = np.float32(slopes[c])
        sB = np.float32(slopes[8 + c])
        dA = np.arange(W_A, dtype=np.float32)[None, :]
        aliA_np = (-sA * (128.0 * dA + 127.0 - jj) + SHIFT).astype(np.float32)
        # e = 8*og - jt in [-7, 24] -> column e + 7
        eB = np.arange(-7, NTT - 7, dtype=np.float32)[None, :]
        aliB_np = (-sB * (128.0 * eB + float(GW - 1) - jj) + SHIFT).astype(np.float32)
        in_maps.append({
            "xT": xT,
            "wq": col_slice(np.asarray(Wq, np.float32), c),
            "wk": col_slice(np.asarray(Wk, np.float32), c),
            "wv": col_slice(np.asarray(Wv, np.float32), c),
            "bq": vec_slice(np.asarray(bq, np.float32), c),
            "bk": vec_slice(np.asarray(bk, np.float32), c),
            "bv": vec_slice(np.asarray(bv, np.float32), c),
            "wo": np.ascontiguousarray(np.concatenate(
                [np.asarray(Wo, np.float32)[64 * c:64 * c + 64, :],
                 np.asarray(Wo, np.float32)[64 * (8 + c):64 * (8 + c) + 64, :]],
                axis=0)).astype(ml_dtypes.bfloat16),
            "aliA": aliA_np,
            "aliB": aliB_np,
            "triA": triA_np,
        })
    return in_maps


LAST_RESULT = None


def kernel(x, Wq, bq, Wk, bk, Wv, bv, Wo, bo, **run_kwargs):
    global LAST_RESULT
    from concourse.bass_utils import run_bass_kernel_spmd

    nc = _build()
    in_maps = shard_inputs(x, Wq, bq, Wk, bk, Wv, bv, Wo, bo)
    res = run_bass_kernel_spmd(nc, in_maps, core_ids=list(range(NCORES)), **run_kwargs)
    LAST_RESULT = res
    total = np.zeros((T, C), dtype=np.float32)
    for r in res.results:
        total += np.asarray(r["out"], dtype=np.float32)
    total += np.asarray(bo, np.float32)[None, :]
    return total.reshape(B, T, C)
